# revision 73
# baseline (speedup 1.0000x reference)
"""Trainium2 Bass kernel for nn_Attention_84327387890534.

Multi-head attention with 1D relative position bias:
  x = x + noise * noise_strength          (folded on host: fp16 cast)
  qkv = x @ w_qkv -> q,k,v per head
  attn = softmax(q k^T * hd^-0.5 + rel_bias[i-j])
  out = (attn @ v) @ w_proj + b_proj

Sharding: data-parallel over batch B=8, one batch per NeuronCore.

Per-core design (fp16 matmul operands, fp32 PSUM accumulation):
  - Host pre-adds noise, casts to fp16, and pre-transposes x into the exact
    SBUF tile layouts (plus a key-reversed copy); every big load is a
    contiguous-per-partition DMA (128 descriptors) split in column pieces
    across the three DMA queues so consumers start on partial data.
  - GLOBAL key reversal (key m <-> n-1-m) for k and v so the per-head
    exp(bias) Toeplitz reads become a single positive-stride Hankel window:
    one [128, 1920] DMA per head from the host-exp'd table.
  - q,k computed transposed (qT/kT = [head*hd + d, n]); v packed per head
    as [v_h | ones] so attn@v also emits softmax row-sums (psum rows 64:128).
  - scores computed transposed S^T[m, p]; exp on ACT (scale folded in),
    bias applied multiplicatively by one DVE mult per key-block.
  - Normalization is pulled off the hot path: per head two DVE copies stash
    the unnormalized out^T and the rowsum row (freeing the attn psum);
    every 4 heads one ACT Ln/Exp pair + a DRAM-expand DMA rebuild the
    replicated 1/rowsum and the aoT rows are scaled in place.
  - attnout^T is exactly the lhsT layout for the projection; b_proj is
    added by DVE against a DMA-broadcast row, then DMA'd out per block.
  - v / qk-pair emission is interleaved into the attention loop as PE
    filler, the next head's first score pair is hoisted before the last
    attn@v, and fin work is deferred one head, so the PE never drains at
    head boundaries (stalls >=100ns halve the PE clock for ~3us).
"""

import sys

import numpy as np
from contextlib import ExitStack

try:
    import concourse.bass as bass
except ImportError:  # pragma: no cover
    sys.path.insert(0, "/opt/trn_rl_repo")
    import concourse.bass as bass

import concourse.tile as tile
from concourse import mybir
from concourse.bass_utils import run_bass_kernel_spmd

F32 = mybir.dt.float32
F16 = mybir.dt.float16

# --- workaround: this walrus build rejects >1 sync-wait command on a single
# TPB_CTRL (Drain) instruction; TileContext's tail drain attaches every
# outstanding semaphore wait to one drain. Split the waits across extra
# drain instructions before the all-engine barrier.
_MAX_WAITS_PER_CTRL = 1


def _split_drain_and_barrier(self, tick_clock, wait_clock):
    import bass_rust
    from concourse.vector_clock import ScopedClock

    nc = self.nc
    drain_inst = nc.sync.drain()
    wait_clock.add_sem_waits(
        drain_inst.ins, ScopedClock({None: tick_clock.global_clock})
    )
    mi = drain_inst.ins
    si = mi.sync_info
    if si is not None and si.on_wait and len(si.on_wait) > _MAX_WAITS_PER_CTRL:
        waits = list(si.on_wait)
        mi.sync_info = bass_rust.SyncInfo(
            on_wait=waits[:_MAX_WAITS_PER_CTRL], on_update=list(si.on_update)
        )
        for i in range(_MAX_WAITS_PER_CTRL, len(waits), _MAX_WAITS_PER_CTRL):
            extra = nc.sync.drain()
            extra.ins.sync_info = bass_rust.SyncInfo(
                on_wait=waits[i:i + _MAX_WAITS_PER_CTRL], on_update=[]
            )

    nc.all_engine_barrier()
    assert self.sems is not None
    popped = nc._tile_sem_poison_stack.pop()
    assert popped is self._sem_poison
    nc.clear_and_free_semaphores(list(self.sems.allocated().values()))
    nc.all_engine_barrier()


tile.TileContext._drain_and_barrier = _split_drain_and_barrier


def _split_multi_waits(nc, max_waits=_MAX_WAITS_PER_CTRL):
    """Walrus here emits at most one sync-wait command per TPB instruction.
    Move excess semaphore waits onto same-engine NoOps inserted just before
    the over-subscribed instruction (identical semantics: engine streams
    are sequential, so the waits still all complete first)."""
    import bass_rust

    for fn in nc.m.functions:
        for bb in fn.blocks:
            out = []
            changed = False
            for inst in bb.instructions:
                si = inst.sync_info
                if si is not None and si.on_wait and len(si.on_wait) > max_waits:
                    waits = list(si.on_wait)
                    extras, keep = waits[:-max_waits], waits[-max_waits:]
                    for i in range(0, len(extras), max_waits):
                        nop = mybir.InstNoOp(
                            name=nc.get_next_instruction_name(), ins=[], outs=[]
                        )
                        nop.engine = inst.engine
                        nop.sync_info = bass_rust.SyncInfo(
                            on_wait=extras[i:i + max_waits], on_update=[]
                        )
                        nc.register_instruction(nop, overwrite=True)
                        out.append(nop)
                    inst.sync_info = bass_rust.SyncInfo(
                        on_wait=keep, on_update=list(si.on_update)
                    )
                    changed = True
                out.append(inst)
            if changed:
                bb.instructions = out
    return nc


# Problem dimensions (hardcoded per harness contract).
B = 8
N = 1024
C = 1024
H = 16
HD = 64
NCORES = 8


def build(n=N, c=C, h=H, hd=HD):
    """Build the single-core SPMD Bass program."""
    assert hd == 64 and c == h * hd and n % 128 == 0 and c % 128 == 0
    ws = n
    tbl_len = 2 * ws - 1
    nb, cb = n // 128, c // 128
    scale = float(hd) ** -0.5
    n512 = [(j0, min(512, n - j0)) for j0 in range(0, n, 512)]
    c512 = [(j0, min(512, c - j0)) for j0 in range(0, c, 512)]
    et_w = n + 128 * (nb - 1)  # 1920: one Hankel window serves all jb

    nc = bass.Bass(trn_type="TRN2")
    # xT16 / xTr16 / wv / wproj arrive from the host pre-blocked in the
    # exact SBUF tile layouts (16KB contiguous per partition -> each load is
    # one DMA with 128 descriptors); ebt is the host-exp'd bias table.
    xt_d = nc.declare_dram_parameter("xT16", [128, 2, cb, 512], F16, isOutput=False)
    xtr_d = nc.declare_dram_parameter("xTr16", [128, 4, cb, 256], F16, isOutput=False)
    wqk_d = nc.declare_dram_parameter("wqk", [cb, 128, cb, 2, 128], F16, isOutput=False)
    wv_d = nc.declare_dram_parameter("wv", [128, 2, cb, 512], F16, isOutput=False)
    wp_d = nc.declare_dram_parameter("wproj", [128, 2, cb, 512], F16, isOutput=False)
    bp_d = nc.declare_dram_parameter("bproj", [1, c], F16, isOutput=False)
    ebt_d = nc.declare_dram_parameter("ebt", [h, tbl_len], F16, isOutput=False)
    out_d = nc.declare_dram_parameter("out", [n, c], F32, isOutput=True)

    with ExitStack() as ctx:
        tc = ctx.enter_context(tile.TileContext(nc))
        const = ctx.enter_context(tc.tile_pool(name="const", bufs=1))



        # Persistent activations. Key order is GLOBALLY REVERSED (m = n-1-j)
        # in kT / vjones so the exp(bias) Hankel reads have positive strides.
        acts = ctx.enter_context(tc.tile_pool(name="acts", bufs=1))
        xT = acts.tile([128, 2, cb, 512], F16, tag="xT")
        xTr = acts.tile([128, 4, cb, 256], F16, tag="xTr")
        qkT = [acts.tile([128, n], F16, tag=f"qkT{i}", name=f"qkT{i}")
               for i in range(2 * cb)]
        vjones = [acts.tile([128, h, 2 * hd], F16, tag=f"vj{i}", name=f"vj{i}")
                  for i in range(nb)]
        aoT = [acts.tile([128, n], F16, tag=f"aoT{i}", name=f"aoT{i}")
               for i in range(cb)]

        p2w = ctx.enter_context(tc.tile_pool(name="wqkp", bufs=3))
        wvp = ctx.enter_context(tc.tile_pool(name="wv", bufs=1))
        atp = ctx.enter_context(tc.tile_pool(name="atp", bufs=4))
        etp = ctx.enter_context(tc.tile_pool(name="etp", bufs=2))
        rcp = ctx.enter_context(tc.tile_pool(name="rcp", bufs=1))
        rgp = ctx.enter_context(tc.tile_pool(name="rgp", bufs=2))
        rrp = ctx.enter_context(tc.tile_pool(name="rrp", bufs=2))
        obp = ctx.enter_context(tc.tile_pool(name="obp", bufs=2))
        dramp = ctx.enter_context(tc.tile_pool(name="dram", bufs=1, space="DRAM"))

        pp = ctx.enter_context(tc.tile_pool(name="pp", bufs=3, space="PSUM"))
        ppo = ctx.enter_context(tc.tile_pool(name="ppo", bufs=1, space="PSUM"))

        ebt_ap = ebt_d[:, :]

        def piece_load(eng, dst, src_d, idx, npieces):
            """Load column-piece idx: contiguous run per partition, 128
            descriptors per DMA."""
            run = cb * n // npieces
            eng.dma_start(
                out=dst[:, idx],
                in_=bass.AP(
                    tensor=src_d[:].tensor,
                    offset=idx * run,
                    ap=[[npieces * run, 128], [1, run]],
                ),
            )

        # weights: qk pairs first on the gpsimd queue (small, needed first)
        wpair_sb = {}

        def load_wpair(g):
            wpair = p2w.tile([128, cb, 2, 128], F16, tag="wpair", name=f"wpair{g}")
            nc.gpsimd.dma_start(
                out=wpair,
                in_=bass.AP(
                    tensor=wqk_d[:].tensor,
                    offset=g * (128 * cb * 2 * 128),
                    ap=[[cb * 2 * 128, 128], [1, cb * 2 * 128]],
                ),
            )
            wpair_sb[g] = wpair

        load_wpair(0)
        load_wpair(1)
        wv_sb = wvp.tile([128, 2, cb, 512], F16, tag="w8", name="wv8")



        # ones halves of vjones (DVE is idle in the prologue; gpsimd memsets
        # would delay the weight DMA dispatch on the same engine)
        for t in range(nb):
            nc.vector.memset(vjones[t][:, :, hd:2 * hd], 1.0)

        # et Hankel prefetch: et(h)[m, f] = exp(tbl)[h, m + f], f in [0, 1920)
        et_tiles = {}

        def load_et(hh, eng=None):
            et = etp.tile([128, et_w], F16, tag="et", name=f"et{hh}")
            (eng or nc.sync).dma_start(
                out=et,
                in_=bass.AP(
                    tensor=ebt_ap.tensor,
                    offset=ebt_ap.offset + hh * tbl_len,
                    ap=[[1, 128], [1, et_w]],
                ),
            )
            et_tiles[hh] = et

        # prologue loads: column pieces spread across the three DMA queues
        # so consumers start on partial data (128 descriptors per DMA)
        piece_load(nc.sync, xT, xt_d, 0, 2)
        piece_load(nc.scalar, xT, xt_d, 1, 2)
        piece_load(nc.gpsimd, wv_sb, wv_d, 0, 2)
        piece_load(nc.gpsimd, wv_sb, wv_d, 1, 2)
        load_et(0, nc.sync)
        load_et(1, nc.scalar)
        piece_load(nc.gpsimd, xTr, xtr_d, 0, 4)
        piece_load(nc.gpsimd, xTr, xtr_d, 1, 4)
        piece_load(nc.sync, xTr, xtr_d, 2, 4)
        piece_load(nc.scalar, xTr, xtr_d, 3, 4)
        # b_proj replicated to all partitions for the DVE output bias-add
        bp_rep = const.tile([128, c], F16, tag="bprep")
        nc.sync.dma_start(
            out=bp_rep,
            in_=bass.AP(tensor=bp_d[:].tensor, offset=0, ap=[[0, 128], [1, c]]),
        )

        def emit_v(t):
            """v_rev block t: psum partitions = reversed keys of block t."""
            ps = pp.tile([128, c], F32, tag="ps", name=f"psv{t}")
            for j0, jl in c512:
                for cc in range(cb):
                    nc.tensor.matmul(
                        ps[:, j0:j0 + jl],
                        xTr[:, t // 2, cc, (t % 2) * 128:(t % 2) * 128 + 128],
                        wv_sb[:, j0 // 512, cc, 0:jl],
                        start=(cc == 0), stop=(cc == cb - 1),
                    )
            nc.vector.tensor_copy(
                vjones[t][:, :, 0:hd],
                ps.rearrange("p (hh d) -> p hh d", hh=h),
            )

        qk_ps = {}

        def emit_qk_half(g, s, ci):
            """One 512-col chunk of qT/kT for pair g; the psum slot is held
            only between the two half-jobs instead of across 16 matmuls."""
            wpair = wpair_sb[g]
            if ci == 0:
                qk_ps[(g, s)] = pp.tile([128, n], F32, tag="ps",
                                        name=f"psqk{g}_{s}")
            ps = qk_ps[(g, s)]
            j0, jl = n512[ci]
            for cc in range(cb):
                nc.tensor.matmul(
                    ps[:, j0:j0 + jl], wpair[:, cc, s, :],
                    xT[:, j0 // 512, cc, 0:jl],
                    start=(cc == 0), stop=(cc == cb - 1),
                )
            if ci == len(n512) - 1:
                del qk_ps[(g, s)]
                if s == 0:
                    nc.vector.tensor_copy(qkT[g], ps)
                else:
                    nc.vector.tensor_copy(
                        qkT[cb + g],
                        bass.AP(
                            tensor=ps[:].tensor,
                            offset=ps[:].offset + n - 1,
                            ap=[ps[:].ap[0], [-1, n]],
                        ),
                    )

        def emit_qk(g, s):
            for ci in range(len(n512)):
                emit_qk_half(g, s, ci)

        # filler jobs interleaved into the attention loop, popped two per
        # head so each holds its psum slot only briefly
        filler = [lambda t=t: emit_v(t) for t in (4, 5, 6, 7)]
        for g in range(2, cb):
            filler.append(lambda g=g: load_wpair(g) or emit_qk_half(g, 0, 0))
            filler.append(lambda g=g: emit_qk_half(g, 0, 1))
            filler.append(lambda g=g: emit_qk_half(g, 1, 0))
            filler.append(lambda g=g: emit_qk_half(g, 1, 1))

        def pop_filler():
            if filler:
                filler.pop(0)()

        head_state = {}

        def setup_head(hh):
            g = hh // 2
            qt_o = (hh * hd) % 128
            head_state[hh] = dict(
                g=g, qt_o=qt_o,
                qT=qkT[g][qt_o:qt_o + hd, :],
                kT=qkT[cb + g][qt_o:qt_o + hd, :],
                et=et_tiles[hh][:],
                at=[None] * 4,
            )

        def emit_fin_last(g, qt_o, poh, hh):
            """Direct fin for the last head (no DRAM round-trip in the tail)."""
            rc = rcp.tile([64, n], F32, tag="rc", name=f"rc{hh}")
            nc.scalar.activation(rc, poh[64:128, :], mybir.ActivationFunctionType.Ln)
            nc.scalar.activation(rc, rc, mybir.ActivationFunctionType.Exp, scale=-1.0)
            nc.vector.tensor_tensor(
                aoT[g][qt_o:qt_o + hd, :], poh[0:hd, :], rc,
                op=mybir.AluOpType.mult,
            )

        # Normalization bookkeeping: per head, two cheap DVE copies pull the
        # unnormalized out^T and the rowsum row out of poh (freeing it for
        # the next head); every 4 heads one Ln/Exp pair plus a DRAM-expand
        # DMA rebuilds the replicated 1/rowsum, and the aoT rows are scaled
        # in place on DVE at fp16 rate.
        FIN_GROUPS = [(0, 4), (4, 8), (8, 12), (12, 15)]
        rstage = dramp.tile([h, n], F16)
        grp_rs = {}

        def emit_copy_rs(g, qt_o, poh, hh):
            gi = min(hh // 4, len(FIN_GROUPS) - 1)
            if gi not in grp_rs:
                grp_rs[gi] = rgp.tile([4, n], F16, tag="rs", name=f"rs{gi}")
            g0, _ = FIN_GROUPS[gi]
            t1 = rgp.tile([1, n], F16, tag="t1", name=f"t1_{hh}")
            nc.vector.tensor_copy(t1, poh[64:65, :])
            # engines can only write partition bases 0/32/64/96: DMA the
            # staged row into group row hh-g0
            nc.sync.dma_start(out=grp_rs[gi][hh - g0:hh - g0 + 1, :], in_=t1)

        def emit_copy_ao(g, qt_o, poh, hh):
            nc.vector.tensor_copy(aoT[g][qt_o:qt_o + hd, :], poh[0:hd, :])

        def emit_copies(g, qt_o, poh, hh):
            emit_copy_rs(g, qt_o, poh, hh)
            emit_copy_ao(g, qt_o, poh, hh)

        grp_ln = {}

        def emit_group_ln(gi):
            g0, g1 = FIN_GROUPS[gi]
            m = g1 - g0
            rln = rgp.tile([4, n], F32, tag="rln", name=f"rln{gi}")
            grp_ln[gi] = rln
            nc.scalar.activation(rln[0:m, :], grp_rs[gi][0:m, :],
                                 mybir.ActivationFunctionType.Ln)

        def emit_group_fin(gi):
            g0, g1 = FIN_GROUPS[gi]
            m = g1 - g0
            rln = grp_ln.pop(gi)
            rcs = rgp.tile([4, n], F16, tag="rcs", name=f"rcs{gi}")
            # fp16 rowsums (values ~1e2..1e4) keep ~5e-4 relative accuracy
            nc.scalar.activation(rcs[0:m, :], rln[0:m, :],
                                 mybir.ActivationFunctionType.Exp, scale=-1.0)
            nc.sync.dma_start(out=rstage[g0:g1, :], in_=rcs[0:m, :])
            for hh in range(g0, g1):
                g = hh // 2
                qt_o = (hh * hd) % 128
                # rr rows must share the aoT slice's base partition
                rr = rrp.tile([128, n], F16, tag="rr", name=f"rr{hh}")
                nc.sync.dma_start(
                    out=rr[qt_o:qt_o + hd, :],
                    in_=bass.AP(
                        tensor=rstage[:].tensor,
                        offset=rstage[:].offset + hh * n,
                        ap=[[0, 64], [1, n]],
                    ),
                )
                ao = aoT[g][qt_o:qt_o + hd, :]
                nc.vector.tensor_tensor(ao, ao, rr[qt_o:qt_o + hd, :],
                                        op=mybir.AluOpType.mult)

        def s_pair(hh, p):
            st = head_state[hh]
            at = atp.tile([128, 2, n], F16, tag="at", name=f"at{hh}_{p}")
            st["at"][p] = at
            for tp in range(2):
                t = 2 * p + tp
                ps = pp.tile([128, n], F32, tag="ps", name=f"pss{hh}_{t}")
                for j0, jl in n512:
                    nc.tensor.matmul(
                        ps[:, j0:j0 + jl],
                        st["kT"][:, t * 128:(t + 1) * 128],
                        st["qT"][:, j0:j0 + jl],
                        start=True, stop=True,
                    )
                nc.scalar.activation(
                    at[:, tp, :], ps, mybir.ActivationFunctionType.Exp,
                    scale=scale,
                )
            # at *= exp(bias) Hankel slice, one DVE op per t so each av half
            # only waits for its own half
            et_ap = st["et"]
            for tp in range(2):
                et2 = bass.AP(
                    tensor=et_ap.tensor,
                    offset=et_ap.offset + 128 * (2 * p + tp),
                    ap=[et_ap.ap[0], [1, n]],
                )
                nc.vector.tensor_tensor(at[:, tp, :], at[:, tp, :], et2,
                                        op=mybir.AluOpType.mult)

        def av_pair(hh, p):
            st = head_state[hh]
            at = st["at"][p]
            poh = st["poh"]
            for tp in range(2):
                t = 2 * p + tp
                for j0, jl in n512:
                    nc.tensor.matmul(
                        poh[:, j0:j0 + jl],
                        vjones[t][:, hh, :],
                        at[:, tp, j0:j0 + jl],
                        start=(t == 0), stop=(t == nb - 1),
                    )

        # prologue PE work: qk first (wpair DMA is tiny; wv's 2MB lands
        # while the qk matmuls run), h0's first scores early so ACT starts.
        emit_qk(0, 0)
        emit_qk(0, 1)
        setup_head(0)
        s_pair(0, 0)
        emit_qk(1, 0)
        emit_qk(1, 1)
        emit_v(0)
        emit_v(1)
        emit_v(2)
        emit_v(3)

        # ---- attention head loop. fin(h-1) is emitted mid-head-h so the ACT
        # queue never head-of-line blocks on the rowsum Ln; the next head's
        # first score pair is hoisted before av_pair(3) so the PE never
        # drains at head boundaries.
        pend_fin = None
        for hh in range(h):
            st = head_state[hh]
            if hh + 1 < h:
                load_et(hh + 1)
            if hh == 10:
                wp_sb = wvp.tile([128, 2, cb, 512], F16, tag="w8", name="wp8")
                piece_load(nc.gpsimd, wp_sb, wp_d, 0, 2)
                piece_load(nc.gpsimd, wp_sb, wp_d, 1, 2)
            st["poh"] = ppo.tile([128, n], F32, tag="po", name=f"po{hh}")

            s_pair(hh, 1)
            if pend_fin is not None:
                # rowsum copy here; the aoT copy after s_pair(2) so the DVE
                # queue delays at most one at-mult per head
                emit_copy_rs(*pend_fin)
            if hh == 0:
                # v(4), v(5) before av_pair(2); v(6), v(7) before av_pair(3)
                pop_filler()
                pop_filler()
            s_pair(hh, 2)
            if pend_fin is not None:
                emit_copy_ao(*pend_fin)
                # split the group-fin ACT burst: Ln this head, Exp+mults next
                if pend_fin[3] in (3, 7, 11):
                    emit_group_ln(pend_fin[3] // 4)
                elif pend_fin[3] in (4, 8, 12):
                    emit_group_fin(pend_fin[3] // 4 - 1)
            # filler here: 2-3 exps are queued, so ACT stays fed through it
            pop_filler()
            av_pair(hh, 0)
            if hh == 0:
                pop_filler()
            s_pair(hh, 3)
            av_pair(hh, 1)
            if hh <= 10:
                # second pop only through h10: stretches the tail of the
                # filler list into heads 11-13, which otherwise idle the PE
                pop_filler()
            av_pair(hh, 2)
            if hh + 1 < h:
                setup_head(hh + 1)
                s_pair(hh + 1, 0)
            av_pair(hh, 3)
            pend_fin = (st["g"], st["qt_o"], st["poh"], hh)
        # tail: group [12,15) (its copies all landed in-loop: head 14's at
        # hh=15), then head 15 directly from psum
        emit_group_ln(3)
        emit_group_fin(3)
        st15 = head_state[h - 1]
        emit_fin_last(st15["g"], st15["qt_o"], st15["poh"], h - 1)



        # ---- proj: out = attnout^T.T @ w_proj + b_proj (bias via DVE add)
        for a in range(nb):
            ps = pp.tile([128, c], F32, tag="ps", name=f"pspr{a}")
            for cc in range(cb):
                for j0, jl in c512:
                    nc.tensor.matmul(
                        ps[:, j0:j0 + jl],
                        aoT[cc][:, a * 128:(a + 1) * 128],
                        wp_sb[:, j0 // 512, cc, 0:jl],
                        start=(cc == 0), stop=(cc == cb - 1),
                    )
            ob = obp.tile([128, c], F32, tag="ob", name=f"ob{a}")
            nc.vector.tensor_tensor(ob, ps, bp_rep, op=mybir.AluOpType.add)
            nc.gpsimd.dma_start(out=out_d[a * 128:(a + 1) * 128, :], in_=ob)

    return _split_multi_waits(nc)


def prep_core_inputs(x2d, noise2d, w_qkv, w_proj, b_proj, tbl, nstr, c=C):
    """Host-side input prep for one core: noise fold, fp16 casts, blocking."""
    cb = c // 128
    xh = (np.asarray(x2d, dtype=np.float32)
          + np.asarray(noise2d, dtype=np.float32)
          * np.float32(np.asarray(nstr, dtype=np.float32))).astype(np.float16)
    # column-piece-major tiles matching the SBUF layouts exactly
    xT = xh.T.reshape(cb, 128, 2, 512).transpose(1, 2, 0, 3)
    xTr = xh[::-1].T.reshape(cb, 128, 4, 256).transpose(1, 2, 0, 3)
    wq = w_qkv[:, :c].astype(np.float16).reshape(cb, 128, cb, 128)
    wk = w_qkv[:, c:2 * c].astype(np.float16).reshape(cb, 128, cb, 128)
    # [pair g, c-row, cc, {q,k}, col]
    wqk = np.ascontiguousarray(
        np.stack([wq.transpose(2, 0, 1, 3), wk.transpose(2, 0, 1, 3)], axis=2)
        .transpose(0, 3, 1, 2, 4)
    )
    def blk(w):  # [c, c] -> [128, 2, cb, 512] matching the SBUF layout
        return np.ascontiguousarray(
            w.astype(np.float16).reshape(cb, 128, 2, 512).transpose(1, 2, 0, 3)
        )

    return dict(
        xT16=np.ascontiguousarray(xT),
        xTr16=np.ascontiguousarray(xTr),
        wqk=wqk,
        wv=blk(w_qkv[:, 2 * c:]),
        wproj=blk(w_proj),
        bproj=np.ascontiguousarray(
            np.asarray(b_proj, dtype=np.float32).astype(np.float16).reshape(1, c)
        ),
        ebt=np.ascontiguousarray(
            np.exp(np.asarray(tbl, dtype=np.float32).T).astype(np.float16)
        ),
    )


_NC_CACHE = {}


def get_nc():
    if "nc" not in _NC_CACHE:
        _NC_CACHE["nc"] = build()
    return _NC_CACHE["nc"]


def kernel(**inputs):
    x = np.asarray(inputs["x"], dtype=np.float32)
    noise = np.asarray(inputs["noise"], dtype=np.float32)
    w_qkv = np.asarray(inputs["w_qkv"], dtype=np.float32)
    w_proj = np.asarray(inputs["w_proj"], dtype=np.float32)
    b_proj = np.asarray(inputs["b_proj"], dtype=np.float32)
    tbl = np.asarray(inputs["rel_bias_table"], dtype=np.float32)
    nstr = np.asarray(inputs["noise_strength"], dtype=np.float32)

    shared = None
    in_maps = []
    for i in range(B):
        m = prep_core_inputs(x[i], noise[i], w_qkv, w_proj, b_proj, tbl, nstr)
        if shared is None:
            shared = {k: v for k, v in m.items() if k not in ("xT16", "xTr16")}
        else:
            for k in shared:
                m[k] = shared[k]
        in_maps.append(m)

    res = run_bass_kernel_spmd(get_nc(), in_maps, list(range(NCORES))).results
    return np.stack([res[i]["out"] for i in range(B)], axis=0).astype(np.float32)


if __name__ == "__main__":
    nc = build()
    print("build ok")


# revision 74
# speedup vs baseline: 1.0090x; 1.0090x over previous
"""Trainium2 Bass kernel for nn_Attention_84327387890534.

Multi-head attention with 1D relative position bias:
  x = x + noise * noise_strength          (folded on host: fp16 cast)
  qkv = x @ w_qkv -> q,k,v per head
  attn = softmax(q k^T * hd^-0.5 + rel_bias[i-j])
  out = (attn @ v) @ w_proj + b_proj

Sharding: data-parallel over batch B=8, one batch per NeuronCore.

Per-core design (fp16 matmul operands, fp32 PSUM accumulation):
  - Host pre-adds noise, casts to fp16, and pre-transposes x into the exact
    SBUF tile layouts (plus a key-reversed copy); every big load is a
    contiguous-per-partition DMA (128 descriptors) split in column pieces
    across the three DMA queues so consumers start on partial data.
  - GLOBAL key reversal (key m <-> n-1-m) for k and v so the per-head
    exp(bias) Toeplitz reads become a single positive-stride Hankel window:
    one [128, 1920] DMA per head from the host-exp'd table.
  - q,k computed transposed (qT/kT = [head*hd + d, n]); v packed per head
    as [v_h | ones] so attn@v also emits softmax row-sums (psum rows 64:128).
  - scores computed transposed S^T[m, p]; exp on ACT (scale folded in),
    bias applied multiplicatively by one DVE mult per key-block.
  - Normalization is pulled off the hot path: per head two DVE copies stash
    the unnormalized out^T and the rowsum row (freeing the attn psum);
    every 4 heads one ACT Ln/Exp pair + a DRAM-expand DMA rebuild the
    replicated 1/rowsum and the aoT rows are scaled in place.
  - attnout^T is exactly the lhsT layout for the projection; b_proj is
    added by DVE against a DMA-broadcast row, then DMA'd out per block.
  - v / qk-pair emission is interleaved into the attention loop as PE
    filler, the next head's first score pair is hoisted before the last
    attn@v, and fin work is deferred one head, so the PE never drains at
    head boundaries (stalls >=100ns halve the PE clock for ~3us).
"""

import sys

import numpy as np
from contextlib import ExitStack

try:
    import concourse.bass as bass
except ImportError:  # pragma: no cover
    sys.path.insert(0, "/opt/trn_rl_repo")
    import concourse.bass as bass

import concourse.tile as tile
from concourse import mybir
from concourse.bass_utils import run_bass_kernel_spmd

F32 = mybir.dt.float32
F16 = mybir.dt.float16

# --- workaround: this walrus build rejects >1 sync-wait command on a single
# TPB_CTRL (Drain) instruction; TileContext's tail drain attaches every
# outstanding semaphore wait to one drain. Split the waits across extra
# drain instructions before the all-engine barrier.
_MAX_WAITS_PER_CTRL = 1


def _split_drain_and_barrier(self, tick_clock, wait_clock):
    import bass_rust
    from concourse.vector_clock import ScopedClock

    nc = self.nc
    drain_inst = nc.sync.drain()
    wait_clock.add_sem_waits(
        drain_inst.ins, ScopedClock({None: tick_clock.global_clock})
    )
    mi = drain_inst.ins
    si = mi.sync_info
    if si is not None and si.on_wait and len(si.on_wait) > _MAX_WAITS_PER_CTRL:
        waits = list(si.on_wait)
        mi.sync_info = bass_rust.SyncInfo(
            on_wait=waits[:_MAX_WAITS_PER_CTRL], on_update=list(si.on_update)
        )
        for i in range(_MAX_WAITS_PER_CTRL, len(waits), _MAX_WAITS_PER_CTRL):
            extra = nc.sync.drain()
            extra.ins.sync_info = bass_rust.SyncInfo(
                on_wait=waits[i:i + _MAX_WAITS_PER_CTRL], on_update=[]
            )

    nc.all_engine_barrier()
    assert self.sems is not None
    popped = nc._tile_sem_poison_stack.pop()
    assert popped is self._sem_poison
    nc.clear_and_free_semaphores(list(self.sems.allocated().values()))
    nc.all_engine_barrier()


tile.TileContext._drain_and_barrier = _split_drain_and_barrier


def _split_multi_waits(nc, max_waits=_MAX_WAITS_PER_CTRL):
    """Walrus here emits at most one sync-wait command per TPB instruction.
    Move excess semaphore waits onto same-engine NoOps inserted just before
    the over-subscribed instruction (identical semantics: engine streams
    are sequential, so the waits still all complete first)."""
    import bass_rust

    for fn in nc.m.functions:
        for bb in fn.blocks:
            out = []
            changed = False
            for inst in bb.instructions:
                si = inst.sync_info
                if si is not None and si.on_wait and len(si.on_wait) > max_waits:
                    waits = list(si.on_wait)
                    extras, keep = waits[:-max_waits], waits[-max_waits:]
                    for i in range(0, len(extras), max_waits):
                        nop = mybir.InstNoOp(
                            name=nc.get_next_instruction_name(), ins=[], outs=[]
                        )
                        nop.engine = inst.engine
                        nop.sync_info = bass_rust.SyncInfo(
                            on_wait=extras[i:i + max_waits], on_update=[]
                        )
                        nc.register_instruction(nop, overwrite=True)
                        out.append(nop)
                    inst.sync_info = bass_rust.SyncInfo(
                        on_wait=keep, on_update=list(si.on_update)
                    )
                    changed = True
                out.append(inst)
            if changed:
                bb.instructions = out
    return nc


# Problem dimensions (hardcoded per harness contract).
B = 8
N = 1024
C = 1024
H = 16
HD = 64
NCORES = 8


def build(n=N, c=C, h=H, hd=HD):
    """Build the single-core SPMD Bass program."""
    assert hd == 64 and c == h * hd and n % 128 == 0 and c % 128 == 0
    ws = n
    tbl_len = 2 * ws - 1
    nb, cb = n // 128, c // 128
    scale = float(hd) ** -0.5
    n512 = [(j0, min(512, n - j0)) for j0 in range(0, n, 512)]
    c512 = [(j0, min(512, c - j0)) for j0 in range(0, c, 512)]
    et_w = n + 128 * (nb - 1)  # 1920: one Hankel window serves all jb

    nc = bass.Bass(trn_type="TRN2")
    # xT16 / xTr16 / wv / wproj arrive from the host pre-blocked in the
    # exact SBUF tile layouts (16KB contiguous per partition -> each load is
    # one DMA with 128 descriptors); ebt is the host-exp'd bias table.
    xt_d = nc.declare_dram_parameter("xT16", [128, 2, cb, 512], F16, isOutput=False)
    xtr_d = nc.declare_dram_parameter("xTr16", [128, 4, cb, 256], F16, isOutput=False)
    wqk_d = nc.declare_dram_parameter("wqk", [cb, 128, cb, 2, 128], F16, isOutput=False)
    wv_d = nc.declare_dram_parameter("wv", [128, 2, cb, 512], F16, isOutput=False)
    wp_d = nc.declare_dram_parameter("wproj", [128, 2, cb, 512], F16, isOutput=False)
    bp_d = nc.declare_dram_parameter("bproj", [1, c], F16, isOutput=False)
    ebt_d = nc.declare_dram_parameter("ebt", [h, tbl_len], F16, isOutput=False)
    out_d = nc.declare_dram_parameter("out", [n, c], F32, isOutput=True)

    with ExitStack() as ctx:
        tc = ctx.enter_context(tile.TileContext(nc))
        const = ctx.enter_context(tc.tile_pool(name="const", bufs=1))



        # Persistent activations. Key order is GLOBALLY REVERSED (m = n-1-j)
        # in kT / vjones so the exp(bias) Hankel reads have positive strides.
        acts = ctx.enter_context(tc.tile_pool(name="acts", bufs=1))
        xT = acts.tile([128, 2, cb, 512], F16, tag="xT")
        xTr = acts.tile([128, 4, cb, 256], F16, tag="xTr")
        qkT = [acts.tile([128, n], F16, tag=f"qkT{i}", name=f"qkT{i}")
               for i in range(2 * cb)]
        vjones = [acts.tile([128, h, 2 * hd], F16, tag=f"vj{i}", name=f"vj{i}")
                  for i in range(nb)]
        aoT = [acts.tile([128, n], F16, tag=f"aoT{i}", name=f"aoT{i}")
               for i in range(cb)]

        p2w = ctx.enter_context(tc.tile_pool(name="wqkp", bufs=3))
        wvp = ctx.enter_context(tc.tile_pool(name="wv", bufs=1))
        atp = ctx.enter_context(tc.tile_pool(name="atp", bufs=4))
        etp = ctx.enter_context(tc.tile_pool(name="etp", bufs=2))
        rcp = ctx.enter_context(tc.tile_pool(name="rcp", bufs=1))
        rgp = ctx.enter_context(tc.tile_pool(name="rgp", bufs=2))
        rrp = ctx.enter_context(tc.tile_pool(name="rrp", bufs=2))
        obp = ctx.enter_context(tc.tile_pool(name="obp", bufs=2))
        dramp = ctx.enter_context(tc.tile_pool(name="dram", bufs=1, space="DRAM"))

        pp = ctx.enter_context(tc.tile_pool(name="pp", bufs=3, space="PSUM"))
        ppo = ctx.enter_context(tc.tile_pool(name="ppo", bufs=1, space="PSUM"))

        ebt_ap = ebt_d[:, :]

        def piece_load(eng, dst, src_d, idx, npieces):
            """Load column-piece idx: contiguous run per partition, 128
            descriptors per DMA."""
            run = cb * n // npieces
            eng.dma_start(
                out=dst[:, idx],
                in_=bass.AP(
                    tensor=src_d[:].tensor,
                    offset=idx * run,
                    ap=[[npieces * run, 128], [1, run]],
                ),
            )

        # weights: qk pairs first on the gpsimd queue (small, needed first)
        wpair_sb = {}

        def load_wpair(g):
            wpair = p2w.tile([128, cb, 2, 128], F16, tag="wpair", name=f"wpair{g}")
            nc.gpsimd.dma_start(
                out=wpair,
                in_=bass.AP(
                    tensor=wqk_d[:].tensor,
                    offset=g * (128 * cb * 2 * 128),
                    ap=[[cb * 2 * 128, 128], [1, cb * 2 * 128]],
                ),
            )
            wpair_sb[g] = wpair

        load_wpair(0)
        load_wpair(1)
        wv_sb = wvp.tile([128, 2, cb, 512], F16, tag="w8", name="wv8")



        # ones halves of vjones (DVE is idle in the prologue; gpsimd memsets
        # would delay the weight DMA dispatch on the same engine)
        for t in range(nb):
            nc.vector.memset(vjones[t][:, :, hd:2 * hd], 1.0)

        # et Hankel prefetch: et(h)[m, f] = exp(tbl)[h, m + f], f in [0, 1920)
        et_tiles = {}

        def load_et(hh, eng=None):
            et = etp.tile([128, et_w], F16, tag="et", name=f"et{hh}")
            (eng or nc.sync).dma_start(
                out=et,
                in_=bass.AP(
                    tensor=ebt_ap.tensor,
                    offset=ebt_ap.offset + hh * tbl_len,
                    ap=[[1, 128], [1, et_w]],
                ),
            )
            et_tiles[hh] = et

        # prologue loads: column pieces spread across the three DMA queues
        # so consumers start on partial data (128 descriptors per DMA)
        piece_load(nc.sync, xT, xt_d, 0, 2)
        piece_load(nc.scalar, xT, xt_d, 1, 2)
        piece_load(nc.gpsimd, wv_sb, wv_d, 0, 2)
        piece_load(nc.gpsimd, wv_sb, wv_d, 1, 2)
        load_et(0, nc.sync)
        load_et(1, nc.scalar)
        piece_load(nc.gpsimd, xTr, xtr_d, 0, 4)
        piece_load(nc.gpsimd, xTr, xtr_d, 1, 4)
        piece_load(nc.sync, xTr, xtr_d, 2, 4)
        piece_load(nc.scalar, xTr, xtr_d, 3, 4)
        # b_proj replicated to all partitions for the DVE output bias-add
        bp_rep = const.tile([128, c], F16, tag="bprep")
        nc.sync.dma_start(
            out=bp_rep,
            in_=bass.AP(tensor=bp_d[:].tensor, offset=0, ap=[[0, 128], [1, c]]),
        )

        def emit_v(t):
            """v_rev block t: psum partitions = reversed keys of block t."""
            ps = pp.tile([128, c], F32, tag="ps", name=f"psv{t}")
            for j0, jl in c512:
                for cc in range(cb):
                    nc.tensor.matmul(
                        ps[:, j0:j0 + jl],
                        xTr[:, t // 2, cc, (t % 2) * 128:(t % 2) * 128 + 128],
                        wv_sb[:, j0 // 512, cc, 0:jl],
                        start=(cc == 0), stop=(cc == cb - 1),
                    )
            nc.vector.tensor_copy(
                vjones[t][:, :, 0:hd],
                ps.rearrange("p (hh d) -> p hh d", hh=h),
            )

        qk_ps = {}

        def emit_qk_half(g, s, ci):
            """One 512-col chunk of qT/kT for pair g; the psum slot is held
            only between the two half-jobs instead of across 16 matmuls."""
            wpair = wpair_sb[g]
            if ci == 0:
                qk_ps[(g, s)] = pp.tile([128, n], F32, tag="ps",
                                        name=f"psqk{g}_{s}")
            ps = qk_ps[(g, s)]
            j0, jl = n512[ci]
            for cc in range(cb):
                nc.tensor.matmul(
                    ps[:, j0:j0 + jl], wpair[:, cc, s, :],
                    xT[:, j0 // 512, cc, 0:jl],
                    start=(cc == 0), stop=(cc == cb - 1),
                )
            if ci == len(n512) - 1:
                del qk_ps[(g, s)]
                if s == 0:
                    nc.vector.tensor_copy(qkT[g], ps)
                else:
                    nc.vector.tensor_copy(
                        qkT[cb + g],
                        bass.AP(
                            tensor=ps[:].tensor,
                            offset=ps[:].offset + n - 1,
                            ap=[ps[:].ap[0], [-1, n]],
                        ),
                    )

        def emit_qk(g, s):
            for ci in range(len(n512)):
                emit_qk_half(g, s, ci)

        # filler jobs interleaved into the attention loop, popped two per
        # head so each holds its psum slot only briefly
        filler = [lambda t=t: emit_v(t) for t in (4, 5, 6, 7)]
        for g in range(2, cb):
            filler.append(lambda g=g: load_wpair(g) or emit_qk_half(g, 0, 0))
            filler.append(lambda g=g: emit_qk_half(g, 0, 1))
            filler.append(lambda g=g: emit_qk_half(g, 1, 0))
            filler.append(lambda g=g: emit_qk_half(g, 1, 1))

        def pop_filler():
            if filler:
                filler.pop(0)()

        head_state = {}

        def setup_head(hh):
            g = hh // 2
            qt_o = (hh * hd) % 128
            head_state[hh] = dict(
                g=g, qt_o=qt_o,
                qT=qkT[g][qt_o:qt_o + hd, :],
                kT=qkT[cb + g][qt_o:qt_o + hd, :],
                et=et_tiles[hh][:],
                at=[None] * 4,
            )

        def emit_fin_last(g, qt_o, poh, hh):
            """Direct fin for the last head (no DRAM round-trip in the tail)."""
            rc = rcp.tile([64, n], F32, tag="rc", name=f"rc{hh}")
            nc.scalar.activation(rc, poh[64:128, :], mybir.ActivationFunctionType.Ln)
            nc.scalar.activation(rc, rc, mybir.ActivationFunctionType.Exp, scale=-1.0)
            nc.vector.tensor_tensor(
                aoT[g][qt_o:qt_o + hd, :], poh[0:hd, :], rc,
                op=mybir.AluOpType.mult,
            )

        # Normalization bookkeeping: per head, two cheap DVE copies pull the
        # unnormalized out^T and the rowsum row out of poh (freeing it for
        # the next head); every 4 heads one Ln/Exp pair plus a DRAM-expand
        # DMA rebuilds the replicated 1/rowsum, and the aoT rows are scaled
        # in place on DVE at fp16 rate.
        FIN_GROUPS = [(0, 4), (4, 8), (8, 12), (12, 15)]
        rstage = dramp.tile([h, n], F16)
        grp_rs = {}

        def emit_copy_rs(g, qt_o, poh, hh):
            gi = min(hh // 4, len(FIN_GROUPS) - 1)
            if gi not in grp_rs:
                grp_rs[gi] = rgp.tile([4, n], F16, tag="rs", name=f"rs{gi}")
            g0, _ = FIN_GROUPS[gi]
            t1 = rgp.tile([1, n], F16, tag="t1", name=f"t1_{hh}")
            nc.vector.tensor_copy(t1, poh[64:65, :])
            # engines can only write partition bases 0/32/64/96: DMA the
            # staged row into group row hh-g0
            nc.sync.dma_start(out=grp_rs[gi][hh - g0:hh - g0 + 1, :], in_=t1)

        def emit_copy_ao(g, qt_o, poh, hh):
            nc.vector.tensor_copy(aoT[g][qt_o:qt_o + hd, :], poh[0:hd, :])

        def emit_copies(g, qt_o, poh, hh):
            emit_copy_rs(g, qt_o, poh, hh)
            emit_copy_ao(g, qt_o, poh, hh)

        grp_ln = {}

        def emit_group_ln(gi):
            g0, g1 = FIN_GROUPS[gi]
            m = g1 - g0
            rln = rgp.tile([4, n], F32, tag="rln", name=f"rln{gi}")
            grp_ln[gi] = rln
            nc.scalar.activation(rln[0:m, :], grp_rs[gi][0:m, :],
                                 mybir.ActivationFunctionType.Ln)

        def emit_group_fin(gi):
            g0, g1 = FIN_GROUPS[gi]
            m = g1 - g0
            rln = grp_ln.pop(gi)
            rcs = rgp.tile([4, n], F16, tag="rcs", name=f"rcs{gi}")
            # fp16 rowsums (values ~1e2..1e4) keep ~5e-4 relative accuracy
            nc.scalar.activation(rcs[0:m, :], rln[0:m, :],
                                 mybir.ActivationFunctionType.Exp, scale=-1.0)
            nc.sync.dma_start(out=rstage[g0:g1, :], in_=rcs[0:m, :])
            for hh in range(g0, g1):
                g = hh // 2
                qt_o = (hh * hd) % 128
                # rr rows must share the aoT slice's base partition
                rr = rrp.tile([128, n], F16, tag="rr", name=f"rr{hh}")
                nc.sync.dma_start(
                    out=rr[qt_o:qt_o + hd, :],
                    in_=bass.AP(
                        tensor=rstage[:].tensor,
                        offset=rstage[:].offset + hh * n,
                        ap=[[0, 64], [1, n]],
                    ),
                )
                ao = aoT[g][qt_o:qt_o + hd, :]
                nc.vector.tensor_tensor(ao, ao, rr[qt_o:qt_o + hd, :],
                                        op=mybir.AluOpType.mult)

        def s_pair(hh, p):
            st = head_state[hh]
            at = atp.tile([128, 2, n], F16, tag="at", name=f"at{hh}_{p}")
            st["at"][p] = at
            for tp in range(2):
                t = 2 * p + tp
                ps = pp.tile([128, n], F32, tag="ps", name=f"pss{hh}_{t}")
                for j0, jl in n512:
                    nc.tensor.matmul(
                        ps[:, j0:j0 + jl],
                        st["kT"][:, t * 128:(t + 1) * 128],
                        st["qT"][:, j0:j0 + jl],
                        start=True, stop=True,
                    )
                nc.scalar.activation(
                    at[:, tp, :], ps, mybir.ActivationFunctionType.Exp,
                    scale=scale,
                )
            # at *= exp(bias) Hankel slice, one DVE op per t so each av half
            # only waits for its own half
            et_ap = st["et"]
            for tp in range(2):
                et2 = bass.AP(
                    tensor=et_ap.tensor,
                    offset=et_ap.offset + 128 * (2 * p + tp),
                    ap=[et_ap.ap[0], [1, n]],
                )
                nc.vector.tensor_tensor(at[:, tp, :], at[:, tp, :], et2,
                                        op=mybir.AluOpType.mult)

        def av_pair(hh, p):
            st = head_state[hh]
            at = st["at"][p]
            poh = st["poh"]
            for tp in range(2):
                t = 2 * p + tp
                for j0, jl in n512:
                    nc.tensor.matmul(
                        poh[:, j0:j0 + jl],
                        vjones[t][:, hh, :],
                        at[:, tp, j0:j0 + jl],
                        start=(t == 0), stop=(t == nb - 1),
                    )

        # prologue PE work: qk first (wpair DMA is tiny; wv's 2MB lands
        # while the qk matmuls run), h0's first scores early so ACT starts.
        emit_qk(0, 0)
        emit_qk(0, 1)
        setup_head(0)
        s_pair(0, 0)
        emit_qk(1, 0)
        emit_qk(1, 1)
        emit_v(0)
        emit_v(1)
        emit_v(2)
        emit_v(3)

        # ---- attention head loop. fin(h-1) is emitted mid-head-h so the ACT
        # queue never head-of-line blocks on the rowsum Ln; the next head's
        # first score pair is hoisted before av_pair(3) so the PE never
        # drains at head boundaries.
        pend_fin = None
        for hh in range(h):
            st = head_state[hh]
            if hh + 1 < h:
                load_et(hh + 1)
            if hh == 10:
                wp_sb = wvp.tile([128, 2, cb, 512], F16, tag="w8", name="wp8")
                piece_load(nc.gpsimd, wp_sb, wp_d, 0, 2)
                piece_load(nc.gpsimd, wp_sb, wp_d, 1, 2)
            st["poh"] = ppo.tile([128, n], F32, tag="po", name=f"po{hh}")

            s_pair(hh, 1)
            if pend_fin is not None:
                # rowsum copy here; the aoT copy after s_pair(2) so the DVE
                # queue delays at most one at-mult per head
                emit_copy_rs(*pend_fin)
            if hh == 0:
                # v(4), v(5) before av_pair(2); v(6), v(7) before av_pair(3)
                pop_filler()
                pop_filler()
            s_pair(hh, 2)
            if pend_fin is not None:
                emit_copy_ao(*pend_fin)
                # split the group-fin ACT burst: Ln this head, Exp+mults next
                if pend_fin[3] in (3, 7, 11):
                    emit_group_ln(pend_fin[3] // 4)
                elif pend_fin[3] in (4, 8, 12):
                    emit_group_fin(pend_fin[3] // 4 - 1)
            # filler here: 2-3 exps are queued, so ACT stays fed through it
            pop_filler()
            av_pair(hh, 0)
            if hh == 0:
                pop_filler()
            s_pair(hh, 3)
            av_pair(hh, 1)
            pop_filler()
            av_pair(hh, 2)
            if hh + 1 < h:
                setup_head(hh + 1)
                s_pair(hh + 1, 0)
            av_pair(hh, 3)
            pend_fin = (st["g"], st["qt_o"], st["poh"], hh)
        # tail: group [12,15) (its copies all landed in-loop: head 14's at
        # hh=15), then head 15 directly from psum
        emit_group_ln(3)
        emit_group_fin(3)
        st15 = head_state[h - 1]
        emit_fin_last(st15["g"], st15["qt_o"], st15["poh"], h - 1)



        # ---- proj: out = attnout^T.T @ w_proj + b_proj (bias via DVE add)
        for a in range(nb):
            ps = pp.tile([128, c], F32, tag="ps", name=f"pspr{a}")
            for cc in range(cb):
                for j0, jl in c512:
                    nc.tensor.matmul(
                        ps[:, j0:j0 + jl],
                        aoT[cc][:, a * 128:(a + 1) * 128],
                        wp_sb[:, j0 // 512, cc, 0:jl],
                        start=(cc == 0), stop=(cc == cb - 1),
                    )
            ob = obp.tile([128, c], F32, tag="ob", name=f"ob{a}")
            nc.vector.tensor_tensor(ob, ps, bp_rep, op=mybir.AluOpType.add)
            nc.gpsimd.dma_start(out=out_d[a * 128:(a + 1) * 128, :], in_=ob)

    return _split_multi_waits(nc)


def prep_core_inputs(x2d, noise2d, w_qkv, w_proj, b_proj, tbl, nstr, c=C):
    """Host-side input prep for one core: noise fold, fp16 casts, blocking."""
    cb = c // 128
    xh = (np.asarray(x2d, dtype=np.float32)
          + np.asarray(noise2d, dtype=np.float32)
          * np.float32(np.asarray(nstr, dtype=np.float32))).astype(np.float16)
    # column-piece-major tiles matching the SBUF layouts exactly
    xT = xh.T.reshape(cb, 128, 2, 512).transpose(1, 2, 0, 3)
    xTr = xh[::-1].T.reshape(cb, 128, 4, 256).transpose(1, 2, 0, 3)
    wq = w_qkv[:, :c].astype(np.float16).reshape(cb, 128, cb, 128)
    wk = w_qkv[:, c:2 * c].astype(np.float16).reshape(cb, 128, cb, 128)
    # [pair g, c-row, cc, {q,k}, col]
    wqk = np.ascontiguousarray(
        np.stack([wq.transpose(2, 0, 1, 3), wk.transpose(2, 0, 1, 3)], axis=2)
        .transpose(0, 3, 1, 2, 4)
    )
    def blk(w):  # [c, c] -> [128, 2, cb, 512] matching the SBUF layout
        return np.ascontiguousarray(
            w.astype(np.float16).reshape(cb, 128, 2, 512).transpose(1, 2, 0, 3)
        )

    return dict(
        xT16=np.ascontiguousarray(xT),
        xTr16=np.ascontiguousarray(xTr),
        wqk=wqk,
        wv=blk(w_qkv[:, 2 * c:]),
        wproj=blk(w_proj),
        bproj=np.ascontiguousarray(
            np.asarray(b_proj, dtype=np.float32).astype(np.float16).reshape(1, c)
        ),
        ebt=np.ascontiguousarray(
            np.exp(np.asarray(tbl, dtype=np.float32).T).astype(np.float16)
        ),
    )


_NC_CACHE = {}


def get_nc():
    if "nc" not in _NC_CACHE:
        _NC_CACHE["nc"] = build()
    return _NC_CACHE["nc"]


def kernel(**inputs):
    x = np.asarray(inputs["x"], dtype=np.float32)
    noise = np.asarray(inputs["noise"], dtype=np.float32)
    w_qkv = np.asarray(inputs["w_qkv"], dtype=np.float32)
    w_proj = np.asarray(inputs["w_proj"], dtype=np.float32)
    b_proj = np.asarray(inputs["b_proj"], dtype=np.float32)
    tbl = np.asarray(inputs["rel_bias_table"], dtype=np.float32)
    nstr = np.asarray(inputs["noise_strength"], dtype=np.float32)

    shared = None
    in_maps = []
    for i in range(B):
        m = prep_core_inputs(x[i], noise[i], w_qkv, w_proj, b_proj, tbl, nstr)
        if shared is None:
            shared = {k: v for k, v in m.items() if k not in ("xT16", "xTr16")}
        else:
            for k in shared:
                m[k] = shared[k]
        in_maps.append(m)

    res = run_bass_kernel_spmd(get_nc(), in_maps, list(range(NCORES))).results
    return np.stack([res[i]["out"] for i in range(B)], axis=0).astype(np.float32)


if __name__ == "__main__":
    nc = build()
    print("build ok")
